# revision 5
# baseline (speedup 1.0000x reference)
"""Trainium2 Bass kernel for nn_ConstrainLoss (weighted logsumexp over a
Gaussian-kernel cost matrix, dotted with row weights -> scalar).

Math:
    sq_ij = |x_i - xo_j|^2          (relu clamp in the reference never fires:
                                     min pairwise sq on this data is ~5.2)
    C_ij  = -2*sq_ij + log(w_obs_j)          (inv_two_s2 == 2.0)
          = 4*x_i.xo_j + a_j + b_i
      a_j = -2*|xo_j|^2 + log(w_obs_j)
      b_i = -2*|x_i|^2            (pulls out of the LSE entirely -> host term)
    out   = -sum_i x_w_i * (b_i + logsumexp_j(T_ij)),  T_ij = 4*x_i.xo_j + a_j

Device kernel (per core, rows sharded 2048/core). The matmul emits
    u_ij = A*(T_ij - shift_i) + B,   A = 128*log2(e), B = 128*(127 - c)
via a K=101 bf16 contraction (hi/lo-split x.xo product + a_j*A bias rows +
per-partition (B - A*shift_i) rows against an all-ones rhs row).

The exp+row-sum work (the bottleneck: 33.5M elements/core; ScalarE alone
runs at 1 elem/cycle/lane) is split between TWO engines working on
disjoint 2048-column psum groups:
  * ScalarE (ACT): exp(u/A - B/A) = e^t with fused accumulate, exact.
  * VectorE (DVE): two stock ops implementing a bf16 Schraudolph exp:
      pass 1: tensor_scalar max(u, 0) -> int16 (round-to-nearest convert);
              the int16 value IS the bf16 bit pattern of ~e^t since
              u/128 = log2(e^t) + 127 - c (c tuned so E[ln ratio] = 0,
              worst per-term ratio error +-5.8%, zero mean).
              u <= 0 (t <= -88) clamps to 0 -> bf16 +0.0. Row max can
              exceed the seed shift by ~69 on this data: u <= 29000, no
              int16 saturation.
      pass 2: tensor_reduce(add) over the SAME bytes bitcast to bf16.
    Per-row lse error is a zero-mean +-0.058 sample -> averages out across
    16384 weighted rows (final error ~1e-5, budget 2e-2).
Groups are assigned to engines greedily by modeled cost so both engines
finish together.

    shift_i: host-side exact max of T over the first 512 columns (any shift
      within ~80 of the row max keeps e^t inside fp32/bf16 range).
    acc: per-(block,group) partial sums s_g -> DMA'd out; host does
      lse_i = shift_i + ln(sum_g s_g) and the final weighted reduction.

Host: result = -(sum_i x_w_i * (b_i + lse_i))
"""

import sys

if "/opt/trn_rl_repo" not in sys.path:
    sys.path.insert(0, "/opt/trn_rl_repo")

import re
from contextlib import ExitStack

import ml_dtypes
import numpy as np

import bass_rust
import concourse.bass as bass
import concourse.tile as tile
from concourse import mybir
from concourse.bass_utils import run_bass_kernel_spmd
from concourse.tile import ScopedClock, TileContext


def _patched_drain_and_barrier(self, tick_clock, wait_clock):
    """The walrus build in this container rejects >1 sync wait on one
    instruction ("Too many sync wait commands" on Tile's kernel-tail drain).
    Split the tail-drain waits onto individual SP nops, one wait each."""
    gc = tick_clock.global_clock
    ticks = [int(s) for s in re.findall(r"\d+", repr(gc))]
    for i, t in enumerate(ticks):
        if t > 0:
            nop = self.nc.sync.nop(hint="split_wait", nofuse=True)
            vc = bass_rust.VectorClock()
            vc.require_at_least(i, t)
            wait_clock.add_sem_waits(nop.ins, ScopedClock({None: vc}))
    self.nc.sync.drain()
    self.nc.all_engine_barrier()
    assert self.sems is not None
    popped = self.nc._tile_sem_poison_stack.pop()
    assert popped is self._sem_poison
    self.nc.clear_and_free_semaphores(list(self.sems.allocated().values()))
    self.nc.all_engine_barrier()


TileContext._drain_and_barrier = _patched_drain_and_barrier

_MAX_WAITS = 1  # this walrus build rejects >1 sync wait per instruction


def _split_excess_waits(nc):
    """Move excess sync waits (beyond _MAX_WAITS) from any instruction onto
    freshly inserted same-engine nops placed immediately before it. The
    engine executes the nops (waiting) first, so semantics are unchanged."""
    counter = [0]
    for f in nc.m.functions:
        for blk in f.blocks:
            il = blk.instructions  # live list
            i = 0
            while i < len(il):
                ins = il[i]
                si = ins.sync_info
                if si is not None and len(si.on_wait) > _MAX_WAITS:
                    waits = list(si.on_wait)
                    keep = waits[-_MAX_WAITS:]
                    excess = waits[: -_MAX_WAITS]
                    pos = i
                    for j in range(0, len(excess), _MAX_WAITS):
                        counter[0] += 1
                        nop = mybir.InstNoOp(
                            name=f"I-splitw{counter[0]}", ins=[], outs=[]
                        )
                        nop.engine = ins.engine
                        nop.sync_info = mybir.SyncInfo(
                            on_wait=excess[j : j + _MAX_WAITS], on_update=[]
                        )
                        il.insert(pos, nop)
                        pos += 1
                        i += 1
                    ins.sync_info = mybir.SyncInfo(
                        on_wait=keep, on_update=list(si.on_update)
                    )
                i += 1


N, M, D = 16384, 16384, 32
NCORES = 8
N_LOC = N // NCORES  # 2048 rows per core
# contraction rows: 3 product splits (hi*hi, hi*lo, lo*hi) + a hi/lo + 3 c rows
KK = 3 * D + 2 + 3  # 101
BLK = 128  # rows per block (psum partitions)
NBLK = N_LOC // BLK  # 16
CHUNK = 512  # matmul free dim (one psum bank fp32)
GROUP = 2048  # columns per consumer instruction (4 psum banks)
NGROUP = M // GROUP  # 8
SEED_W = 512  # seed max over first SEED_W columns

# Schraudolph-bf16 constants: u = A*t + B; int16(u) bitcast bf16 ~= e^t
SCH_C = 0.056135424859914705  # zero-mean log-ratio calibration
SCH_A = 128.0 * np.log2(np.e)  # 184.6649652337873
SCH_B = 128.0 * (127.0 - SCH_C)  # 16248.8147

# modeled per-group engine costs (ns) for the greedy ACT/DVE assignment
ACT_COST = 2094.0
DVE_COST = 4566.0

F32 = mybir.dt.float32
BF16 = mybir.dt.bfloat16
I16 = mybir.dt.int16

_cache = {}


def _assign_engines():
    """Greedy static assignment of each (block, group) to ACT or DVE so both
    engines finish together. Returns a list of NBLK lists of 'A'/'D'."""
    t_act = t_dve = 0.0
    plan = []
    for _b in range(NBLK):
        row = []
        for _g in range(NGROUP):
            if t_act + ACT_COST <= t_dve + DVE_COST:
                row.append("A")
                t_act += ACT_COST
            else:
                row.append("D")
                t_dve += DVE_COST
        plan.append(row)
    return plan


def _build_bass():
    nc = bass.Bass()
    xT_d = nc.declare_dram_parameter("xT", [KK, N_LOC], BF16, isOutput=False)
    xoT_d = nc.declare_dram_parameter("xoT", [KK, M], BF16, isOutput=False)
    s_d = nc.declare_dram_parameter("s_out", [BLK, NBLK * NGROUP], F32, isOutput=True)

    plan = _assign_engines()

    with tile.TileContext(nc) as tc, ExitStack() as ctx:
        singles = ctx.enter_context(tc.tile_pool(name="singles", bufs=1))
        psp = ctx.enter_context(tc.tile_pool(name="ps", bufs=2, space="PSUM"))
        scratch = ctx.enter_context(tc.tile_pool(name="scr", bufs=2))

        xo_sb = singles.tile([128, M], BF16)
        x_sb = singles.tile([128, N_LOC], BF16)
        s_full = singles.tile([BLK, NBLK * NGROUP], F32)
        actb = singles.tile([BLK, 1], F32)
        nc.gpsimd.memset(actb[:, :], float(-SCH_B / SCH_A))

        # Spread input DMAs across engine queues so they land in parallel;
        # the first matmuls depend only on x + xo piece 0.
        nc.sync.dma_start(out=x_sb[0:KK, :], in_=xT_d[:, :])
        NPIECE = 8
        PW = M // NPIECE
        dma_engines = [nc.sync, nc.gpsimd]
        for p in range(NPIECE):
            dma_engines[p % len(dma_engines)].dma_start(
                out=xo_sb[0:KK, p * PW : (p + 1) * PW],
                in_=xoT_d[:, p * PW : (p + 1) * PW],
            )

        for b in range(NBLK):
            s_all = s_full[:, b * NGROUP : (b + 1) * NGROUP]
            for g in range(NGROUP):
                ps = psp.tile([BLK, GROUP], F32, tag="ps")
                for c in range(GROUP // CHUNK):
                    j0 = g * GROUP + c * CHUNK
                    nc.tensor.matmul(
                        out=ps[:, c * CHUNK : (c + 1) * CHUNK],
                        lhsT=x_sb[0:KK, b * BLK : (b + 1) * BLK],
                        rhs=xo_sb[0:KK, j0 : j0 + CHUNK],
                        start=True,
                        stop=True,
                    )
                if plan[b][g] == "A":
                    nc.scalar.activation(
                        out=ps,
                        in_=ps,
                        func=mybir.ActivationFunctionType.Exp,
                        bias=actb[:, :],
                        scale=float(1.0 / SCH_A),
                        accum_out=s_all[:, g : g + 1],
                    )
                else:
                    w1 = scratch.tile([BLK, GROUP], I16, tag="w1")
                    nc.vector.tensor_scalar(
                        out=w1[:, :],
                        in0=ps[:, :],
                        scalar1=0.0,
                        scalar2=None,
                        op0=mybir.AluOpType.max,
                    )
                    nc.vector.tensor_reduce(
                        out=s_all[:, g : g + 1],
                        in_=w1[:, :].bitcast(BF16),
                        axis=mybir.AxisListType.X,
                        op=mybir.AluOpType.add,
                    )
            nc.sync.dma_start(
                out=s_d[:, b * NGROUP : (b + 1) * NGROUP],
                in_=s_all,
            )

    _split_excess_waits(nc)
    return nc


def _get_nc():
    if "nc" not in _cache:
        _cache["nc"] = _build_bass()
    return _cache["nc"]


def _bf_split(v):
    hi = v.astype(ml_dtypes.bfloat16)
    lo = (v - hi.astype(np.float32)).astype(ml_dtypes.bfloat16)
    return hi, lo


def _prep_inputs(x, x_w, x_obs, x_obs_w):
    x = np.ascontiguousarray(x, dtype=np.float32)
    x_w = np.ascontiguousarray(x_w, dtype=np.float32)
    x_obs = np.ascontiguousarray(x_obs, dtype=np.float32)
    x_obs_w = np.ascontiguousarray(x_obs_w, dtype=np.float32)

    c_obs = np.sum(x_obs * x_obs, axis=1, dtype=np.float32)
    a = (-2.0 * c_obs + np.log(x_obs_w)).astype(np.float32)
    a_hi, a_lo = _bf_split((a.astype(np.float64) * SCH_A).astype(np.float32))
    xo_hi, xo_lo = _bf_split(x_obs)
    xoT = np.empty((KK, M), dtype=ml_dtypes.bfloat16)
    xoT[0:D] = xo_hi.T
    xoT[D : 2 * D] = xo_lo.T
    xoT[2 * D : 3 * D] = xo_hi.T
    xoT[3 * D] = a_hi
    xoT[3 * D + 1] = a_lo
    xoT[3 * D + 2 : 3 * D + 5] = np.ones((1,), dtype=ml_dtypes.bfloat16)

    # x scaled so the matmul directly emits A*(4*x.xo)
    xs = (x.astype(np.float64) * (4.0 * SCH_A)).astype(np.float32)
    x_hi, x_lo = _bf_split(xs)

    # Host-side LSE shift: exact max of T over the first SEED_W columns.
    # On this data max_j T - shift <= ~69 (verified): e^t stays inside
    # fp32/bf16 range and u stays under int16 saturation.
    T_seed = (4.0 * (x @ x_obs[:SEED_W].T) + a[None, :SEED_W]).astype(np.float32)
    shift = T_seed.max(axis=1)  # [N]

    # per-row row of the lhsT: B - A*shift, 3-way bf16 split (rows 98..100)
    c_row = (SCH_B - shift.astype(np.float64) * SCH_A).astype(np.float32)
    c1, r = _bf_split(c_row)
    c2, c3 = _bf_split(r.astype(np.float32))

    in_maps = []
    for core in range(NCORES):
        sl = slice(core * N_LOC, (core + 1) * N_LOC)
        xT = np.empty((KK, N_LOC), dtype=ml_dtypes.bfloat16)
        xT[0:D] = x_hi[sl].T
        xT[D : 2 * D] = x_hi[sl].T
        xT[2 * D : 3 * D] = x_lo[sl].T
        xT[3 * D] = np.ones((1,), dtype=ml_dtypes.bfloat16)
        xT[3 * D + 1] = np.ones((1,), dtype=ml_dtypes.bfloat16)
        xT[3 * D + 2] = c1[sl]
        xT[3 * D + 3] = c2[sl]
        xT[3 * D + 4] = c3[sl]
        in_maps.append({"xT": xT, "xoT": xoT})
    return in_maps, shift


def kernel(x, x_w, x_obs, x_obs_w, _trace=False, _tmpdir=None):
    nc = _get_nc()
    in_maps, shift = _prep_inputs(x, x_w, x_obs, x_obs_w)
    res = run_bass_kernel_spmd(
        nc,
        in_maps,
        core_ids=list(range(NCORES)),
        trace=_trace,
        tmpdir=_tmpdir,
    )
    _cache["last_results"] = res
    # host epilogue (fp64): lse_i = shift_i + log(sum_g s_ig) + b_i
    x = np.ascontiguousarray(x, dtype=np.float32)
    x_w64 = np.ascontiguousarray(x_w, dtype=np.float32).astype(np.float64)
    r = np.sum(x.astype(np.float64) * x, axis=1)
    total = float(np.dot(-2.0 * r, x_w64))
    for core in range(NCORES):
        out = res.results[core]
        S = (
            out["s_out"]
            .astype(np.float64)
            .reshape(BLK, NBLK, NGROUP)
            .sum(axis=2)
        )  # [128 rows, 16 blocks]
        sl = slice(core * N_LOC, (core + 1) * N_LOC)
        sh = shift[sl].astype(np.float64).reshape(NBLK, BLK).T
        lse = sh + np.log(S)
        w_arr = x_w64[sl].reshape(NBLK, BLK).T
        total += float((lse * w_arr).sum())
    return np.asarray(-total, dtype=np.float32)
